# revision 1
# baseline (speedup 1.0000x reference)
"""Windowed attention + FC Trainium2 kernel.

Problem: B=8, N=16384 (128x128 token grid), C=256, 8 heads x d=32,
8x8 windows (256 windows of 64 tokens per batch element), softmax attention
within each window, then out @ W_fc.T.

Sharding: data-parallel over batch -- one batch element per NeuronCore
(8 cores), W_fc replicated. No collectives. Within a core, windows are
processed in horizontally-adjacent pairs (128 tokens per pair).

Self-contained: builds the Bass program at first call, runs it on cores 0-7
via run_bass_kernel_spmd, reassembles the full [8, 16384, 256] output.
"""

import json

import numpy as np

import concourse.bass as bass
import concourse.tile as tile
from concourse import mybir
from concourse.masks import make_identity

F32 = mybir.dt.float32
AF = mybir.ActivationFunctionType

N, C, H, D, GS = 16384, 256, 8, 32, 64
SCALE = D ** -0.5
REARR_QK = ("(a i b j) c -> a i (b j) c", dict(a=16, i=8, b=16, j=8))
REARR_V = ("(a i b j) c -> a b i j c", dict(a=16, i=8, b=16, j=8))

# ---------------------------------------------------------------------------
# Workaround: this container's walrus build encodes at most ONE sem wait per
# instruction ("Too many sync wait commands"), but Tile attaches all required
# waits to the consuming instruction. Rewrite the BIR at serialization time:
# any instruction with N>1 waits is preceded by N-1 same-engine NoOps
# carrying one wait each (sequencers execute in order, AND-semantics kept).
# ---------------------------------------------------------------------------
_MAX_WAITS = 1


def _split_waits(mod: dict) -> dict:
    counter = [0]

    def mk_nop(inst, waits):
        counter[0] += 1
        return {
            "name": f"{inst['name']}-wspill{counter[0]}",
            "opcode": "NoOp",
            "engine": inst.get("engine"),
            "ins": [],
            "outs": [],
            "sync_info": {"on_wait": waits, "on_update": []},
            "debug": inst.get("debug", 0),
        }

    for fn in mod.get("functions", []):
        for blk in fn.get("blocks", []):
            insts = blk.get("instructions")
            if not insts:
                continue
            out = []
            for inst in insts:
                si = inst.get("sync_info") or {}
                waits = si.get("on_wait") or []
                if len(waits) > _MAX_WAITS:
                    extra, keep = waits[:-_MAX_WAITS], waits[-_MAX_WAITS:]
                    for i in range(0, len(extra), _MAX_WAITS):
                        out.append(mk_nop(inst, extra[i:i + _MAX_WAITS]))
                    si = dict(si)
                    si["on_wait"] = keep
                    inst = dict(inst)
                    inst["sync_info"] = si
                out.append(inst)
            blk["instructions"] = out
    return mod


_patch_done = False


def _apply_patch():
    global _patch_done
    if _patch_done:
        return
    orig = bass.Bass.to_json_bytes

    def patched(self, *a, **kw):
        mod = json.loads(orig(self, *a, **kw))
        return json.dumps(_split_waits(mod)).encode()

    bass.Bass.to_json_bytes = patched
    _patch_done = True


# ---------------------------------------------------------------------------
# Kernel builder (see wa_core.py history for layout derivation):
#  - scores S^T per (head, window): K=32 M=64 N=64 matmuls with PE 4x row
#    tiling; concurrent row tiles MUST write different PSUM banks (verified
#    HW constraint) -> S4 [128, 2048] spans 4 banks, row tile r = h%4 uses
#    bank r, window jw selects out partitions 64jw.
#  - one-op exp over the strided 4-bank AP; E cols: h -> 128*(h%4)+64*(h//4)
#  - O_u + denominators (ones-stationary matmul, replicated across the 32
#    d-partitions) per window jw into bank jw of od [128, 1024];
#    od col = 512*w + 128*b + 64*t2 + q (b: 0=O 1=D).
#  - X = O_u * recip(D) on DVE; FC with X as stationary, out [tok, co].
# ---------------------------------------------------------------------------
def build_kernel(nc: bass.Bass, wr_list=tuple(range(16))):
    q = nc.dram_tensor("q", [N, C], F32, kind="ExternalInput")
    k = nc.dram_tensor("k", [N, C], F32, kind="ExternalInput")
    v = nc.dram_tensor("v", [N, C], F32, kind="ExternalInput")
    w = nc.dram_tensor("w_fc", [C, C], F32, kind="ExternalInput")
    out = nc.dram_tensor("out", [N, C], F32, kind="ExternalOutput")

    qv = q.rearrange(REARR_QK[0], **REARR_QK[1])
    kv = k.rearrange(REARR_QK[0], **REARR_QK[1])
    vv = v.rearrange(REARR_V[0], **REARR_V[1])
    ov = out.rearrange(REARR_V[0], **REARR_V[1])

    with tile.TileContext(nc) as tc:
        with (
            tc.tile_pool(name="constp", bufs=1) as constp,
            tc.tile_pool(name="iop", bufs=3) as iop,
            tc.tile_pool(name="midp", bufs=2) as midp,
            tc.tile_pool(name="ps_tp", bufs=1, space="PSUM") as ps_tp,
            tc.tile_pool(name="ps_s", bufs=1, space="PSUM") as ps_s,
            tc.tile_pool(name="ps_od", bufs=1, space="PSUM") as ps_od,
            tc.tile_pool(name="ps_fc", bufs=1, space="PSUM") as ps_fc,
        ):
            ident = constp.tile([128, 128], F32)
            make_identity(nc, ident)
            ones32 = constp.tile([128, 32], F32)
            nc.gpsimd.memset(ones32[:], 1.0)

            wt = [constp.tile([128, 256], F32, name=f"wt{t}") for t in range(2)]
            for coh in range(2):
                wn = constp.tile([128, 256], F32, tag="wn", bufs=2,
                                 name=f"wn{coh}")
                nc.sync.dma_start(out=wn[:], in_=w[coh * 128:(coh + 1) * 128, :])
                wp = ps_tp.tile([128, 512], F32, tag="tp", name=f"wp{coh}")
                for cih in range(2):
                    nc.tensor.transpose(wp[:, cih * 128:(cih + 1) * 128],
                                        wn[:, cih * 128:(cih + 1) * 128],
                                        ident[:])
                for cih in range(2):
                    nc.scalar.copy(wt[cih][:, coh * 128:(coh + 1) * 128],
                                   wp[:, cih * 128:(cih + 1) * 128])

            for wr in wr_list:
                for wpr in range(8):
                    wc0 = 2 * wpr
                    qn = iop.tile([128, 256], F32, tag="qn")
                    kn = iop.tile([128, 256], F32, tag="kn")
                    vn = iop.tile([128, 256], F32, tag="vn")
                    nc.sync.dma_start(
                        out=qn[:], in_=qv[wr, :, wc0 * 8:(wc0 + 2) * 8, :])
                    nc.sync.dma_start(
                        out=kn[:], in_=kv[wr, :, wc0 * 8:(wc0 + 2) * 8, :])
                    for jw in range(2):
                        nc.sync.dma_start(out=vn[64 * jw:64 * jw + 64, :],
                                          in_=vv[wr, wc0 + jw])

                    tp = ps_tp.tile([128, 512], F32, tag="tp")
                    for ch in range(2):
                        nc.tensor.transpose(tp[:, ch * 128:(ch + 1) * 128],
                                            qn[:, ch * 128:(ch + 1) * 128],
                                            ident[:])
                        nc.tensor.transpose(
                            tp[:, 256 + ch * 128:256 + (ch + 1) * 128],
                            kn[:, ch * 128:(ch + 1) * 128], ident[:])
                    qt = midp.tile([128, 256], F32, tag="qt")
                    kt = midp.tile([128, 256], F32, tag="kt")
                    rr_dst = ("p (w i j) -> p i w j", dict(w=2, i=8, j=8))
                    rr_src = ("p (i w j) -> p i w j", dict(i=8, w=2, j=8))
                    for t in range(2):
                        c0 = 128 * t
                        nc.scalar.copy(
                            qt[:, c0:c0 + 128].rearrange(rr_dst[0],
                                                         **rr_dst[1]),
                            tp[:, c0:c0 + 128].rearrange(rr_src[0],
                                                         **rr_src[1]))
                        nc.vector.tensor_copy(
                            kt[:, c0:c0 + 128].rearrange(rr_dst[0],
                                                         **rr_dst[1]),
                            tp[:, 256 + c0:256 + c0 + 128].rearrange(
                                rr_src[0], **rr_src[1]))

                    s4 = ps_s.tile([128, 2048], F32, tag="s4")
                    for h in range(H):
                        t2, r = h // 4, h % 4
                        for jw in range(2):
                            cs = 128 * t2 + 64 * jw
                            nc.tensor.matmul(
                                s4[64 * jw:64 * jw + 64,
                                   512 * r + 64 * t2:512 * r + 64 * t2 + 64],
                                kt[32 * r:32 * r + 32, cs:cs + 64],
                                qt[32 * r:32 * r + 32, cs:cs + 64],
                                start=True, stop=True,
                                tile_position=(32 * r, 64 * jw))

                    E = midp.tile([128, 512], F32, tag="E")
                    nc.scalar.activation(
                        E.rearrange("p (r c) -> p r c", r=4, c=128),
                        s4.rearrange("p (r c) -> p r c",
                                     r=4, c=512)[:, :, 0:128],
                        AF.Exp, scale=SCALE)

                    od = ps_od.tile([128, 1024], F32, tag="od")
                    for h in range(H):
                        t2, c = h // 4, h % 4
                        ec = 128 * (h % 4) + 64 * (h // 4)
                        for jw in range(2):
                            ev = E[64 * jw:64 * jw + 64, ec:ec + 64]
                            o0 = 512 * jw + 64 * t2
                            nc.tensor.matmul(
                                od[32 * c:32 * c + 32, o0:o0 + 64],
                                vn[64 * jw:64 * jw + 64, 32 * h:32 * h + 32],
                                ev, start=True, stop=True,
                                tile_position=(64 * jw, 32 * c))
                            nc.tensor.matmul(
                                od[32 * c:32 * c + 32, o0 + 128:o0 + 192],
                                ones32[64 * jw:64 * jw + 64, :],
                                ev, start=True, stop=True,
                                tile_position=(64 * jw, 32 * c))

                    R = midp.tile([128, 256], F32, tag="R")
                    nc.vector.reciprocal(
                        R.rearrange("p (w c) -> p w c", w=2, c=128),
                        od.rearrange("p (w u b c) -> p w u b c",
                                     w=2, u=2, b=2, c=128)[:, :, 0, 1, :])
                    X = midp.tile([128, 256], F32, tag="X")
                    nc.vector.tensor_mul(
                        X.rearrange("p (t w q) -> p w t q", t=2, w=2, q=64),
                        od.rearrange("p (w u b t q) -> p w u b t q",
                                     w=2, u=2, b=2, t=2, q=64)[:, :, 0, 0],
                        R.rearrange("p (w t q) -> p w t q", w=2, t=2, q=64))

                    fc = ps_fc.tile([128, 512], F32, tag="fc")
                    nc.tensor.matmul(fc[:, 0:256], X[:, 0:128], wt[0][:],
                                     start=True, stop=False)
                    nc.tensor.matmul(fc[:, 0:256], X[:, 128:256], wt[1][:],
                                     start=False, stop=True)
                    Y = iop.tile([128, 256], F32, tag="Y")
                    nc.scalar.copy(Y[:], fc[:, 0:256])
                    for jw in range(2):
                        nc.sync.dma_start(out=ov[wr, wc0 + jw],
                                          in_=Y[64 * jw:64 * jw + 64, :])

    return nc


_nc_cache = None


def _get_nc():
    global _nc_cache
    if _nc_cache is None:
        _apply_patch()
        nc = bass.Bass()
        build_kernel(nc)
        _nc_cache = nc
    return _nc_cache


def kernel(q, k, v, W_fc):
    from concourse.bass_utils import run_bass_kernel_spmd

    q = np.ascontiguousarray(np.asarray(q, dtype=np.float32))
    k = np.ascontiguousarray(np.asarray(k, dtype=np.float32))
    v = np.ascontiguousarray(np.asarray(v, dtype=np.float32))
    W_fc = np.ascontiguousarray(np.asarray(W_fc, dtype=np.float32))
    B = q.shape[0]
    assert B == 8 and q.shape[1:] == (N, C)

    nc = _get_nc()
    in_maps = [
        {"q": q[b], "k": k[b], "v": v[b], "w_fc": W_fc} for b in range(B)
    ]
    res = run_bass_kernel_spmd(nc, in_maps, core_ids=list(range(B)))
    return np.stack([res.results[b]["out"] for b in range(B)], axis=0)


# revision 4
# speedup vs baseline: 15985.2540x; 15985.2540x over previous
"""Windowed attention + FC Trainium2 kernel.

Problem: B=8, N=16384 (128x128 token grid), C=256, 8 heads x d=32,
8x8 windows (256 windows of 64 tokens per batch element), softmax attention
within each window, then out @ W_fc.T.

Sharding: data-parallel over batch -- one batch element per NeuronCore
(8 cores), W_fc replicated. No collectives. Within a core, windows are
processed in horizontally-adjacent pairs (128 tokens per pair).

Self-contained: builds the Bass program at first call, runs it on cores 0-7
via run_bass_kernel_spmd, reassembles the full [8, 16384, 256] output.
"""

import json

import numpy as np

import concourse.bass as bass
import concourse.tile as tile
from concourse import mybir
from concourse.masks import make_identity

F32 = mybir.dt.float32
AF = mybir.ActivationFunctionType

N, C, H, D, GS = 16384, 256, 8, 32, 64
SCALE = D ** -0.5
REARR_QK = ("(a i b j) c -> a i (b j) c", dict(a=16, i=8, b=16, j=8))
REARR_V = ("(a i b j) c -> a b i j c", dict(a=16, i=8, b=16, j=8))

# ---------------------------------------------------------------------------
# Workaround: this container's walrus build encodes at most ONE sem wait per
# instruction ("Too many sync wait commands"), but Tile attaches all required
# waits to the consuming instruction. Rewrite the BIR at serialization time:
# any instruction with N>1 waits is preceded by N-1 same-engine NoOps
# carrying one wait each (sequencers execute in order, AND-semantics kept).
# ---------------------------------------------------------------------------
_MAX_WAITS = 1


def _split_waits(mod: dict) -> dict:
    counter = [0]

    def mk_nop(inst, waits):
        counter[0] += 1
        return {
            "name": f"{inst['name']}-wspill{counter[0]}",
            "opcode": "NoOp",
            "engine": inst.get("engine"),
            "ins": [],
            "outs": [],
            "sync_info": {"on_wait": waits, "on_update": []},
            "debug": inst.get("debug", 0),
        }

    for fn in mod.get("functions", []):
        for blk in fn.get("blocks", []):
            insts = blk.get("instructions")
            if not insts:
                continue
            out = []
            for inst in insts:
                si = inst.get("sync_info") or {}
                waits = si.get("on_wait") or []
                if len(waits) > _MAX_WAITS:
                    extra, keep = waits[:-_MAX_WAITS], waits[-_MAX_WAITS:]
                    for i in range(0, len(extra), _MAX_WAITS):
                        out.append(mk_nop(inst, extra[i:i + _MAX_WAITS]))
                    si = dict(si)
                    si["on_wait"] = keep
                    inst = dict(inst)
                    inst["sync_info"] = si
                out.append(inst)
            blk["instructions"] = out
    return mod


_patch_done = False


def _apply_patch():
    global _patch_done
    if _patch_done:
        return
    orig = bass.Bass.to_json_bytes

    def patched(self, *a, **kw):
        mod = json.loads(orig(self, *a, **kw))
        return json.dumps(_split_waits(mod)).encode()

    bass.Bass.to_json_bytes = patched
    _patch_done = True


# ---------------------------------------------------------------------------
# Kernel builder (see wa_core.py history for layout derivation):
#  - scores S^T per (head, window): K=32 M=64 N=64 matmuls with PE 4x row
#    tiling; concurrent row tiles MUST write different PSUM banks (verified
#    HW constraint) -> S4 [128, 2048] spans 4 banks, row tile r = h%4 uses
#    bank r, window jw selects out partitions 64jw.
#  - one-op exp over the strided 4-bank AP; E cols: h -> 128*(h%4)+64*(h//4)
#  - O_u + denominators (ones-stationary matmul, replicated across the 32
#    d-partitions) per window jw into bank jw of od [128, 1024];
#    od col = 512*w + 128*b + 64*t2 + q (b: 0=O 1=D).
#  - X = O_u * recip(D) on DVE; FC with X as stationary, out [tok, co].
# ---------------------------------------------------------------------------
def build_kernel(nc: bass.Bass, wr_list=tuple(range(16)), repeat=None):
    q = nc.dram_tensor("q", [N, C], F32, kind="ExternalInput")
    k = nc.dram_tensor("k", [N, C], F32, kind="ExternalInput")
    v = nc.dram_tensor("v", [N, C], F32, kind="ExternalInput")
    w = nc.dram_tensor("w_fc", [C, C], F32, kind="ExternalInput")
    out = nc.dram_tensor("out", [N, C], F32, kind="ExternalOutput")

    qv = q.rearrange(REARR_QK[0], **REARR_QK[1])
    kv = k.rearrange(REARR_QK[0], **REARR_QK[1])
    vv = v.rearrange(REARR_V[0], **REARR_V[1])
    ov = out.rearrange(REARR_V[0], **REARR_V[1])

    with tile.TileContext(nc) as tc:
        with (
            tc.tile_pool(name="constp", bufs=1) as constp,
            tc.tile_pool(name="iop", bufs=3) as iop,
            tc.tile_pool(name="midp", bufs=2) as midp,
            tc.tile_pool(name="ps_tp", bufs=1, space="PSUM") as ps_tp,
            tc.tile_pool(name="ps_s", bufs=1, space="PSUM") as ps_s,
            tc.tile_pool(name="ps_od", bufs=1, space="PSUM") as ps_od,
            tc.tile_pool(name="ps_fc", bufs=1, space="PSUM") as ps_fc,
        ):
            ident = constp.tile([128, 128], F32)
            make_identity(nc, ident)
            ones32 = constp.tile([128, 32], F32)
            nc.gpsimd.memset(ones32[:], 1.0)

            wt = [constp.tile([128, 256], F32, name=f"wt{t}") for t in range(2)]
            for coh in range(2):
                wn = constp.tile([128, 256], F32, tag="wn", bufs=2,
                                 name=f"wn{coh}")
                nc.sync.dma_start(out=wn[:], in_=w[coh * 128:(coh + 1) * 128, :])
                wp = ps_tp.tile([128, 512], F32, tag="tp", name=f"wp{coh}")
                for cih in range(2):
                    nc.tensor.transpose(wp[:, cih * 128:(cih + 1) * 128],
                                        wn[:, cih * 128:(cih + 1) * 128],
                                        ident[:])
                for cih in range(2):
                    nc.scalar.copy(wt[cih][:, coh * 128:(coh + 1) * 128],
                                   wp[:, cih * 128:(cih + 1) * 128])

            import contextlib
            rep_ctx = (tc.For_i(0, repeat, 1) if repeat
                       else contextlib.nullcontext())
            with rep_ctx:
                _body(nc, tc, wr_list, qv, kv, vv, ov, ident, ones32, wt,
                      iop, midp, ps_tp, ps_s, ps_od, ps_fc)

    return nc


def _body(nc, tc, wr_list, qv, kv, vv, ov, ident, ones32, wt,
          iop, midp, ps_tp, ps_s, ps_od, ps_fc):
    if True:
            for wr in wr_list:
                for wpr in range(8):
                    wc0 = 2 * wpr
                    qn = iop.tile([128, 256], F32, tag="qn")
                    kn = iop.tile([128, 256], F32, tag="kn")
                    vn = iop.tile([128, 256], F32, tag="vn")
                    nc.sync.dma_start(
                        out=qn[:], in_=qv[wr, :, wc0 * 8:(wc0 + 2) * 8, :])
                    nc.sync.dma_start(
                        out=kn[:], in_=kv[wr, :, wc0 * 8:(wc0 + 2) * 8, :])
                    for jw in range(2):
                        nc.sync.dma_start(out=vn[64 * jw:64 * jw + 64, :],
                                          in_=vv[wr, wc0 + jw])

                    tp = ps_tp.tile([128, 512], F32, tag="tp")
                    for ch in range(2):
                        nc.tensor.transpose(tp[:, ch * 128:(ch + 1) * 128],
                                            qn[:, ch * 128:(ch + 1) * 128],
                                            ident[:])
                        nc.tensor.transpose(
                            tp[:, 256 + ch * 128:256 + (ch + 1) * 128],
                            kn[:, ch * 128:(ch + 1) * 128], ident[:])
                    qt = midp.tile([128, 256], F32, tag="qt")
                    kt = midp.tile([128, 256], F32, tag="kt")
                    rr_dst = ("p (w i j) -> p i w j", dict(w=2, i=8, j=8))
                    rr_src = ("p (i w j) -> p i w j", dict(i=8, w=2, j=8))
                    for t in range(2):
                        c0 = 128 * t
                        nc.scalar.copy(
                            qt[:, c0:c0 + 128].rearrange(rr_dst[0],
                                                         **rr_dst[1]),
                            tp[:, c0:c0 + 128].rearrange(rr_src[0],
                                                         **rr_src[1]))
                        nc.vector.tensor_copy(
                            kt[:, c0:c0 + 128].rearrange(rr_dst[0],
                                                         **rr_dst[1]),
                            tp[:, 256 + c0:256 + c0 + 128].rearrange(
                                rr_src[0], **rr_src[1]))

                    s4 = ps_s.tile([128, 2048], F32, tag="s4")
                    for h in range(H):
                        t2, r = h // 4, h % 4
                        for jw in range(2):
                            cs = 128 * t2 + 64 * jw
                            nc.tensor.matmul(
                                s4[64 * jw:64 * jw + 64,
                                   512 * r + 64 * t2:512 * r + 64 * t2 + 64],
                                kt[32 * r:32 * r + 32, cs:cs + 64],
                                qt[32 * r:32 * r + 32, cs:cs + 64],
                                start=True, stop=True,
                                tile_position=(32 * r, 64 * jw))

                    E = midp.tile([128, 512], F32, tag="E")
                    nc.scalar.activation(
                        E.rearrange("p (r c) -> p r c", r=4, c=128),
                        s4.rearrange("p (r c) -> p r c",
                                     r=4, c=512)[:, :, 0:128],
                        AF.Exp, scale=SCALE)

                    od = ps_od.tile([128, 1024], F32, tag="od")
                    for h in range(H):
                        t2, c = h // 4, h % 4
                        ec = 128 * (h % 4) + 64 * (h // 4)
                        for jw in range(2):
                            ev = E[64 * jw:64 * jw + 64, ec:ec + 64]
                            o0 = 512 * jw + 64 * t2
                            nc.tensor.matmul(
                                od[32 * c:32 * c + 32, o0:o0 + 64],
                                vn[64 * jw:64 * jw + 64, 32 * h:32 * h + 32],
                                ev, start=True, stop=True,
                                tile_position=(64 * jw, 32 * c))
                            nc.tensor.matmul(
                                od[32 * c:32 * c + 32, o0 + 128:o0 + 192],
                                ones32[64 * jw:64 * jw + 64, :],
                                ev, start=True, stop=True,
                                tile_position=(64 * jw, 32 * c))

                    R = midp.tile([128, 256], F32, tag="R")
                    nc.vector.reciprocal(
                        R.rearrange("p (w c) -> p w c", w=2, c=128),
                        od.rearrange("p (w u b c) -> p w u b c",
                                     w=2, u=2, b=2, c=128)[:, :, 0, 1, :])
                    X = midp.tile([128, 256], F32, tag="X")
                    nc.vector.tensor_mul(
                        X.rearrange("p (t w q) -> p w t q", t=2, w=2, q=64),
                        od.rearrange("p (w u b t q) -> p w u b t q",
                                     w=2, u=2, b=2, t=2, q=64)[:, :, 0, 0],
                        R.rearrange("p (w t q) -> p w t q", w=2, t=2, q=64))

                    fc = ps_fc.tile([128, 512], F32, tag="fc")
                    nc.tensor.matmul(fc[:, 0:256], X[:, 0:128], wt[0][:],
                                     start=True, stop=False)
                    nc.tensor.matmul(fc[:, 0:256], X[:, 128:256], wt[1][:],
                                     start=False, stop=True)
                    Y = iop.tile([128, 256], F32, tag="Y")
                    nc.scalar.copy(Y[:], fc[:, 0:256])
                    for jw in range(2):
                        nc.sync.dma_start(out=ov[wr, wc0 + jw],
                                          in_=Y[64 * jw:64 * jw + 64, :])


_nc_cache = None


def _get_nc():
    global _nc_cache
    if _nc_cache is None:
        _apply_patch()
        nc = bass.Bass()
        build_kernel(nc)
        _nc_cache = nc
    return _nc_cache


def kernel(q, k, v, W_fc):
    from concourse.bass_utils import run_bass_kernel_spmd

    q = np.ascontiguousarray(np.asarray(q, dtype=np.float32))
    k = np.ascontiguousarray(np.asarray(k, dtype=np.float32))
    v = np.ascontiguousarray(np.asarray(v, dtype=np.float32))
    W_fc = np.ascontiguousarray(np.asarray(W_fc, dtype=np.float32))
    B = q.shape[0]
    assert B == 8 and q.shape[1:] == (N, C)

    nc = _get_nc()
    in_maps = [
        {"q": q[b], "k": k[b], "v": v[b], "w_fc": W_fc} for b in range(B)
    ]
    res = run_bass_kernel_spmd(nc, in_maps, core_ids=list(range(B)))
    return np.stack([res.results[b]["out"] for b in range(B)], axis=0)
